# revision 39
# baseline (speedup 1.0000x reference)
"""ColBERT MaxSim contrastive loss on 8 Trainium2 NeuronCores.

scores[b, c] = (1/q_len[b]) * sum_n max_s <q[b, n, :], d[c, s, :]>
loss = CE(scores / T, labels=arange(B)), mean reduction.

Sharding: data-parallel over the *doc* batch dim (columns of the score
matrix). Each core holds the full query set plus its 8-doc shard
(both fp8e4m3, scaled x16), computes its (B_global, B_local) =
(64, 8) block of per-token reductions, and the host performs the ln,
token-pair sums, gather, and the tiny 64x64 CE reduction.

Compute structure:
  The kernel is drain-bound: every PSUM sim element needs exactly one
  first touch by ACT (~1.30 us per [128,1024] tile incl. accumulator
  read) or DVE (~1.17 us); 128 tiles split 60/68 keep both engines at
  ~79.5 us busy, which is the wall. Both touches are terminal:
   - direct tiles: DVE reduce_max straight off PSUM -> maxes (fp16);
   - LSE tiles: token max via sharp logsumexp,
       max_s x_s ~= 1 + (1/beta) ln sum_s exp(beta (x_s - 1)),
     one ACTIVATE(Exp, accum_out->S) per tile, ln done on host.
  The LSE/direct choice is per (query-token-group, doc) with 8 or 7
  LSE groups per doc, so each score row keeps a near-uniform LSE bias
  that cancels in the row-wise softmax of the CE loss (measured rel
  err ~1e-3 at beta=128, tolerance 2e-2).

Efficiency notes:
  - Matmuls are emitted group-major (all 16 N=512 matmuls of a query
    group back-to-back, same stationary lhs); a BIR post-pass drops
    Ldweights whose weight AP matches the previous one on the PE
    queue: 17 weight loads instead of 257 (~28 us of PE time).
  - Both embedding tensors travel as fp8e4m3 scaled x16 (1.25 MB
    total instead of 2.5 MB); fp8 matmuls run at the same 1 col/cycle.
    The DVE maxes come out x256 (host divides); the ACT Exp folds the
    1/256 into its scale operand. Sim-checked rel err 1.06e-3.
  - Inputs load in chunks as separate tiles (qT 2, dT 4) ordered by
    first use; each dma_start costs ~0.6 us of serialized
    descriptor-gen on the SP sequencer, so chunk count balances start
    latency against issue time. First matmul at ~12 us of which ~7 us
    is fixed framework preamble.
  - 8 dummy matmuls on memset data run during the DMA wait so the PE
    p-state (full clock only after ~3 us continuous execution) ramps
    before the real work.
  - PSUM holds 4 rotating [128,1024] tiles (8 banks); drains alternate
    engines via the per-(g,doc) parity pattern, with flipped groups
    interleaved mid-group to avoid drain-engine bubbles, and the last
    group LSE-first so the S DMA overlaps the trailing reduces.
  - Raw S (fp32) and maxes (fp16) DMA out; no on-device ln/sel-matmul
    tail. Host does ln(S)/beta + pair sums + CE in float64.

Host: S/maxes blocks -> scores (64, 64) -> q_len scaling -> CE loss.
"""

import json

import numpy as np

import concourse.bass as bass
import concourse.mybir as mybir
import concourse.tile as tile
from concourse.bass_utils import run_bass_kernel_spmd

B = 64          # queries (= docs, contrastive batch)
NQ = 32         # tokens per query
ND = 1024       # tokens per doc
D = 128         # embedding dim
NCORES = 8
CL = B // NCORES  # docs per core
TEMPERATURE = 0.02
NORMALIZE_SCORES = True
BETA = 128.0    # LSE sharpness

F32 = mybir.dt.float32
F16 = mybir.dt.float16
BF16 = mybir.dt.bfloat16
F8 = mybir.dt.float8e4
DSCALE = 16.0       # host scale for each fp8 embedding side
SIMSCALE = DSCALE * DSCALE   # resulting scale on the sim values

NG = (B * NQ) // 128        # 16 query groups of 4 queries
NSETS = NG * CL             # 128 (query group, doc) tiles

# Per-(group, doc) drain assignment. LSE iff (g+t) even, except one
# flipped group on odd docs (8 or 7 LSE groups per doc) -> ACT 60
# tiles / DVE 68 tiles, matching the engines' measured drain rates
# (1.30 vs 1.17 us/tile at spec clock). The +-1 per-doc LSE count adds
# a small column-consistent bias (sim-checked rel err ~1e-3, tol 2e-2).
# Flip placement: one flipped group per odd doc.
_FLIPS = {(2 * t + 1, t) for t in (1, 3, 5, 7)}
ASSIGN = {}
_lse_i = 0
_dir_i = 0
for _g in range(NG):
    for _t in range(CL):
        if (_g + _t) % 2 == 0 and (_g, _t) not in _FLIPS:
            ASSIGN[(_g, _t)] = ("lse", _lse_i)
            _lse_i += 1
        else:
            ASSIGN[(_g, _t)] = ("dir", _dir_i)
            _dir_i += 1
NLSE = _lse_i               # 60
NDIR = _dir_i               # 68

# qT column chunks (in groups) / dT column chunks (in docs): separate
# tiles so early matmuls only wait on the chunks they read. Each
# dma_start costs ~0.6 us of serialized descriptor-gen on the issuing
# sequencer, so chunk count trades start latency against issue time.
# Arrival has to keep ahead of the drain pace (~0.62 us/doc-tile).
QT_CHUNKS = [(0, 2), (2, 16)]             # [start_group, end_group)
DT_CHUNKS = [(0, 2), (2, 4), (4, 6), (6, 8)]  # [start_doc, end_doc)

# Within-group doc emission order: interleave DVE- and ACT-drained docs
# so both drain engines stay busy even in the flipped (5 DVE / 3 ACT)
# groups. Even groups lead with their LSE doc (keeps doc 0 first in
# group 0, matching DMA arrival order). The tail group runs its LSE
# docs early so the trailing S/maxes DMAs overlap the last reduces.
# (A fully boundary-chained alternation was measured: it removes the
# ~0.6 us group-boundary bubbles but each back-to-back PSUM reduce
# slows ~19 ns from read/write bank contention -- net zero.)
GROUP_ORDER = {}
for _g in range(NG):
    _dirs = [t for t in range(CL) if ASSIGN[(_g, t)][0] == "dir"]
    _lses = [t for t in range(CL) if ASSIGN[(_g, t)][0] == "lse"]
    _a, _b = (_lses, _dirs) if (_g % 2 == 0 and len(_lses) >= len(_dirs)) \
        else (_dirs, _lses)
    _order = []
    _ia = _ib = 0
    for _i in range(CL):
        if (_i % 2 == 0 and _ia < len(_a)) or _ib >= len(_b):
            _order.append(_a[_ia]); _ia += 1
        else:
            _order.append(_b[_ib]); _ib += 1
    if len(_dirs) == 5 and _g != NG - 1:
        # flip groups: bury the extra DVE doc mid-group
        _order = [_dirs[0], _lses[0], _dirs[1], _lses[1],
                  _dirs[2], _dirs[3], _lses[2], _dirs[4]]
    if _g == NG - 1:
        _order = []
        for _i in range(len(_lses)):
            _order += [_lses[_i], _dirs[_i]]
        _order += _dirs[len(_lses):]
    GROUP_ORDER[_g] = _order
    assert sorted(_order) == list(range(CL))

def _dedup_ldweights_json(bir_bytes: bytes) -> bytes:
    """Drop PE Ldweights whose operands match the previous Ldweights on
    the same queue (the PE array keeps the stationary operand between
    matmuls). Any waits/updates on a dropped load move to the next PE
    instruction; _split_waits_json legalizes multi-wait results."""
    bir = json.loads(bir_bytes)
    for f in bir["functions"]:
        for blk in f["blocks"]:
            out = []
            last_sig = None
            pend_w: list = []
            pend_u: list = []
            for ins in blk["instructions"]:
                if ins.get("engine") != "PE":
                    out.append(ins)
                    continue
                op = ins["opcode"]
                if op == "Ldweights":
                    sig = json.dumps(
                        [ins.get("ins"), ins.get("perf_mode"),
                         ins.get("is_transpose"), ins.get("tile_position"),
                         ins.get("tile_size")], sort_keys=True)
                    if sig == last_sig:
                        si = ins.get("sync_info") or {}
                        pend_w += si.get("on_wait") or []
                        pend_u += si.get("on_update") or []
                        continue
                    last_sig = sig
                elif op != "Matmult":
                    # sequencer-only PE instrs (Drain/EventSemaphore/...)
                    # don't touch the array; keep the cached weights.
                    pass
                if pend_w or pend_u:
                    si = ins.get("sync_info") or {"on_wait": [], "on_update": []}
                    si["on_wait"] = pend_w + (si.get("on_wait") or [])
                    si["on_update"] = (si.get("on_update") or []) + pend_u
                    ins["sync_info"] = si
                    pend_w, pend_u = [], []
                out.append(ins)
            assert not pend_w and not pend_u
            blk["instructions"] = out
    return json.dumps(bir).encode()


def _split_waits_json(bir_bytes: bytes) -> bytes:
    """Walrus in this toolchain rejects >1 sem-wait per instruction; split
    extra waits onto preceding Drains."""
    bir = json.loads(bir_bytes)
    for f in bir["functions"]:
        for blk in f["blocks"]:
            fixed = []
            for ins in blk["instructions"]:
                si = ins.get("sync_info") or {}
                waits = si.get("on_wait") or []
                if len(waits) > 1:
                    for i, w in enumerate(waits[:-1]):
                        fixed.append({
                            "debug": ins.get("debug", 0),
                            "engine": ins["engine"],
                            "ins": [],
                            "is_reset_sema": False,
                            "name": f'{ins["name"]}-wsplit{i}',
                            "opcode": "Drain",
                            "outs": [],
                            "sync_info": {"on_update": [], "on_wait": [w]},
                        })
                    si["on_wait"] = waits[-1:]
                    ins["sync_info"] = si
                fixed.append(ins)
            blk["instructions"] = fixed
    return json.dumps(bir).encode()


def _trim_end_json(bir_bytes: bytes) -> bytes:
    """Slim the epilogue: (1) drop end-block waits implied transitively
    by the output-DMA completions (input-DMA lanes and the Pool memset
    sem), (2) drop the second, redundant all-engine barrier round (the
    first round already restores the barrier semaphores to zero)."""
    bir = json.loads(bir_bytes)
    for f in bir["functions"]:
        for blk in f["blocks"]:
            if not blk.get("name", "").endswith("_end"):
                continue
            ins_list = blk["instructions"]
            # output DMA lanes = two highest DMAHW lane ids waited on
            lanes = []
            for ins in ins_list:
                for w in (ins.get("sync_info") or {}).get("on_wait") or []:
                    n = w.get("ant_name", "")
                    if n.startswith("DMAHW"):
                        lanes.append(n)
            keep_lanes = set(sorted(set(lanes))[-2:])
            out = []
            seen_pool_isa = False
            for ins in ins_list:
                si = ins.get("sync_info") or {}
                waits = si.get("on_wait") or []
                if seen_pool_isa and ins["opcode"] in ("Drain",
                                                       "EventSemaphore"):
                    continue    # second barrier round
                if (ins["opcode"] == "Drain" and len(waits) == 1
                        and not (si.get("on_update") or [])):
                    n = waits[0].get("ant_name", "")
                    if (n.startswith("DMAHW") and n not in keep_lanes) \
                            or n.startswith("Pool_"):
                        continue
                if ins["opcode"] == "ISA" and ins.get("engine") == "Pool":
                    seen_pool_isa = True
                out.append(ins)
            blk["instructions"] = out
    return json.dumps(bir).encode()


def _trim_main_barrier_json(bir_bytes: bytes) -> bytes:
    """Drop the second all-engine barrier round in the 'main' block
    (engines only do engine-local register init there; every
    cross-engine hazard in the tile block is semaphore-guarded, and
    the barrier sems net to zero with or without the round). The
    first, runtime-released round lives in an earlier block and is
    untouched."""
    bir = json.loads(bir_bytes)
    for f in bir["functions"]:
        for blk in f["blocks"]:
            if blk.get("name") != "main":
                continue
            out = []
            for ins in blk["instructions"]:
                if ins["opcode"] in ("Drain", "EventSemaphore"):
                    si = ins.get("sync_info") or {}
                    refs = [x.get("ant_name", "") for x in
                            (si.get("on_wait") or []) +
                            (si.get("on_update") or [])]
                    if any(r.startswith("barrier_") for r in refs):
                        continue
                out.append(ins)
            blk["instructions"] = out
    return json.dumps(bir).encode()


def _patch_nc(nc):
    orig = nc.to_json_bytes

    def patched(*a, **k):
        return _trim_main_barrier_json(_trim_end_json(
            _split_waits_json(_dedup_ldweights_json(orig(*a, **k)))))

    nc.to_json_bytes = patched
    return nc


def build_nc():
    """Build the per-core Bass program (SPMD: every core runs this; only
    the data in its "dT" shard differs)."""
    nc = bass.Bass("TRN2", target_bir_lowering=False, debug=False,
                   num_devices=NCORES)
    qT_dram = nc.dram_tensor("qT", [128, NG * 128], F8,
                             kind="ExternalInput").ap()
    dT_dram = nc.dram_tensor("dT", [128, CL * 1024], F8,
                             kind="ExternalInput").ap()
    S_dram = nc.dram_tensor("S_out", [128, NLSE], F32,
                            kind="ExternalOutput").ap()
    mx_dram = nc.dram_tensor("mx_out", [128, NDIR], F16,
                             kind="ExternalOutput").ap()

    with tile.TileContext(nc) as tc:
        with (
            tc.tile_pool(name="prep", bufs=1) as prep,
            tc.tile_pool(name="mm", bufs=4, space="PSUM") as psum_pool,
        ):
            # ---- inputs: chunked straight loads, wide descriptors,
            # issued in first-use order: the drains consume a doc every
            # ~0.62 us once rolling, so the whole 1 MB fp8 doc shard
            # must land within ~5 us of the first matmul. ----
            qT_tiles = [prep.tile([128, 256], F8, name="qt0"),
                        prep.tile([128, 1792], F8, name="qt1")]
            dT_tiles = [prep.tile([128, 2048], F8, name="dt0"),
                        prep.tile([128, 2048], F8, name="dt1"),
                        prep.tile([128, 2048], F8, name="dt2"),
                        prep.tile([128, 2048], F8, name="dt3")]
            nc.sync.dma_start(qT_tiles[0][:], qT_dram[:, 0:256])
            nc.sync.dma_start(dT_tiles[0][:], dT_dram[:, 0:2048])
            nc.sync.dma_start(dT_tiles[1][:], dT_dram[:, 2048:4096])
            nc.sync.dma_start(dT_tiles[2][:], dT_dram[:, 4096:6144])
            nc.sync.dma_start(dT_tiles[3][:], dT_dram[:, 6144:8192])
            nc.sync.dma_start(qT_tiles[1][:], qT_dram[:, 256:2048])

            def lhs_ap(g):
                for (s, e), t in zip(QT_CHUNKS, qT_tiles):
                    if s <= g < e:
                        return t[:, (g - s) * 128:(g - s + 1) * 128]
                raise AssertionError

            def rhs_ap(t):
                for (s, e), tl in zip(DT_CHUNKS, dT_tiles):
                    if s <= t < e:
                        return tl[:, (t - s) * 1024:(t - s + 1) * 1024]
                raise AssertionError

            # exp bias tile: exp(BETA * x - BETA)
            ebias = prep.tile([128, 1], F32, tag="eb", name="eb")
            nc.gpsimd.memset(ebias[:], -BETA)

            # PE warmup: the tensor engine p-state reaches full clock
            # only after ~3 us of continuous execution. Burn that ramp
            # on dummy matmuls (zeroed SBUF -> a rotation slot later
            # overwritten with start=True) while the input DMAs are
            # still in flight, so the real matmuls start warm.
            warm = psum_pool.tile([128, 1024], F32, tag="pa", name="warm")
            wsrc = prep.tile([128, 640], BF16, tag="wsrc", name="wsrc")
            nc.vector.memset(wsrc[:], 0.0)
            for _ in range(8):
                nc.tensor.matmul(warm[:, 0:512], wsrc[:, 0:128],
                                 wsrc[:, 128:640], start=True, stop=True)

            # maxes holds the direct token maxes (fp16, DSCALE-scaled);
            # S the LSE exp-sums (fp32).
            maxes = prep.tile([128, NDIR], F16, name="maxes")
            S = prep.tile([128, NLSE], F32, tag="S", name="S")
            scratch = prep.tile([128, 1024], BF16, tag="scr", name="scr")

            # ---- main loop: group-major (one stationary lhs per
            # group, deduped to one Ldweights); drains alternate
            # ACT/DVE via the per-(g, doc) parity assignment through
            # the 4-slot PSUM rotation. ----
            for g in range(NG):
                lhs = lhs_ap(g)
                for t in GROUP_ORDER[g]:
                    rhs = rhs_ap(t)
                    pa = psum_pool.tile([128, 1024], F32, tag="pa", name="pa")
                    nc.tensor.matmul(pa[:, 0:512], lhs, rhs[:, 0:512],
                                     start=True, stop=True)
                    nc.tensor.matmul(pa[:, 512:1024], lhs, rhs[:, 512:1024],
                                     start=True, stop=True)
                    kind, i = ASSIGN[(g, t)]
                    if kind == "lse":
                        nc.scalar.activation(
                            scratch[:], pa[:],
                            mybir.ActivationFunctionType.Exp,
                            bias=ebias[:], scale=BETA / SIMSCALE,
                            accum_out=S[:, i:i + 1])
                    else:
                        nc.vector.reduce_max(maxes[:, i:i + 1], pa[:],
                                             axis=mybir.AxisListType.X)

            # ship S and the token maxes raw; the host does the ln,
            # the token-pair sums, and the CE loss. The S DMA only
            # waits the last EXP, overlapping the trailing reduces.
            nc.sync.dma_start(S_dram, S[:])
            nc.sync.dma_start(mx_dram, maxes[:])

    nc.finalize()
    return _patch_nc(nc)


_NC = None


def _get_nc():
    global _NC
    if _NC is None:
        _NC = build_nc()
    return _NC


def make_in_maps(q, d):
    """Host prep: fp16 cast + the 128-block transposes.

    qT[:, g*128 + j] = q_flat[16j + g, :] (q_flat = tokens row-major);
    dT doc block t holds d[t, 8*pp + x, :] at column t*1024 + x*128 + pp.
    """
    import ml_dtypes
    q8 = (np.asarray(q) * DSCALE).astype(
        ml_dtypes.float8_e4m3).reshape(B * NQ, D)
    qT = np.ascontiguousarray(
        q8.reshape(128, 16, D).transpose(2, 1, 0).reshape(D, NG * 128))
    d8 = (np.asarray(d) * DSCALE).astype(ml_dtypes.float8_e4m3)
    in_maps = []
    for k in range(NCORES):
        ds = d8[CL * k:CL * (k + 1)]              # (8, 1024, 128)
        # (doc, 128 pp, 8 x, 128 dd) -> (dd, doc, x, pp)
        dTk = ds.reshape(CL, 128, 8, D).transpose(3, 0, 2, 1)
        dTk = np.ascontiguousarray(dTk.reshape(D, CL * 8 * 128))
        in_maps.append({"qT": qT, "dT": dTk})
    return in_maps


def assemble_loss(outs, q):
    """Host tail: per-core raw S (128, NLSE) and maxes (128, NDIR) ->
    scores -> CE loss. Partition p of column (g, t) is token 16p+g of
    query (16p+g)//32; token pairs (2m, 2m+1) belong to query m.
    LSE: tokmax ~= 1 + ln(S)/BETA; direct: tokmax = maxes/SIMSCALE."""
    scores = np.zeros((B, B), np.float64)
    for k in range(NCORES):
        S, mx = outs[k]
        lnS = np.log(np.asarray(S, np.float64))          # (128, NLSE)
        mxv = np.asarray(mx, np.float64) / SIMSCALE      # (128, NDIR)
        acc = np.zeros((B, CL), np.float64)
        for (g, t), (kind, i) in ASSIGN.items():
            if kind == "lse":
                tok = lnS[:, i] / BETA + 1.0
            else:
                tok = mxv[:, i]
            # partition p -> query (16p+g)//32
            q_of_p = (16 * np.arange(128) + g) // 32
            np.add.at(acc[:, t], q_of_p, tok)
        scores[:, CL * k:CL * (k + 1)] = acc
    if NORMALIZE_SCORES:
        q_len = (np.asarray(q)[:, :, 0] != 0).sum(axis=1).astype(np.float64)
        scores = scores / q_len[:, None]
    logits = scores / TEMPERATURE
    m = logits.max(axis=1, keepdims=True)
    logz = m[:, 0] + np.log(np.exp(logits - m).sum(axis=1))
    loss = -(np.diag(logits) - logz).mean()
    return np.float32(loss)


def kernel(query_embeddings, doc_embeddings):
    q = np.ascontiguousarray(np.asarray(query_embeddings, dtype=np.float32))
    d = np.ascontiguousarray(np.asarray(doc_embeddings, dtype=np.float32))
    nc = _get_nc()
    in_maps = make_in_maps(q, d)
    res = run_bass_kernel_spmd(nc, in_maps, core_ids=list(range(NCORES)))
    outs = [(res.results[k]["S_out"], res.results[k]["mx_out"])
            for k in range(NCORES)]
    return assemble_loss(outs, q)
